# revision 1
# baseline (speedup 1.0000x reference)
"""Batch-hard triplet loss (CrossCameraTripletLoss) on 8 TRN2 NeuronCores.

Strategy (data-parallel over anchor rows, label-sorted + per-core rotated):
  - Host: stable-sort rows by label so each class is a contiguous candidate
    range [S, E); shard 1024 sorted anchors per core. Each core receives the
    full candidate set ROTATED left by its anchor offset, so row-block rb's
    class windows always live in the fixed column band [128*rb-64, 128*rb+192)
    - the same program works on every core (SPMD).
  - Device: PSUM tiles hold -d2(i,j) = 2<x_i,x_j> - |x_j|^2 - |x_i|^2 via two
    accumulating bf16 matmuls (dot + K=4 aux carrying hi/lo-split norms).
    Hard-negative mining: plain tensor_reduce(max) over -d2 on the row pieces
    outside the band (constant offsets), plus a masked band contribution
    (-d2 - BIG*ind, ind = host-computed class-window indicator).
    Hard-positive mining: +d2 band via two small matmuls, masked to the class
    window with ind, reduce(max).
  - Tail: loss_i = relu(sqrt(relu(d2_pos)) - sqrt(relu(d2_neg)) + margin),
    validity-weighted, reduced on-device to one scalar per core; host sums 8
    partials / valid count.
  - pairwise_distance's eps (1e-6 on the difference) shifts distances by
    ~1e-7 relative - far below tolerance - so mined values are used directly.
"""

import sys

sys.path.insert(0, "/opt/trn_rl_repo")

import numpy as np
import ml_dtypes

import concourse.bacc as bacc
import concourse.mybir as mybir
import concourse.tile as tile
from concourse.bass_utils import run_bass_kernel_spmd

F32 = mybir.dt.float32
BF16 = mybir.dt.bfloat16
BF = ml_dtypes.bfloat16
NEG_INF = -3.4e38
MARGIN = 0.2
BIG = 1.0e7
PAD_SQ = 3.0e4

N, D, NCORES = 8192, 128, 8
M = N // NCORES          # anchors per core
RB = M // 128            # row blocks per core
CH = 2048                # chunk width
NCH = N // CH
BW = 256                 # band width
BOFF = 64                # band margin left of the block's first anchor

TRACE = False
LAST_RESULTS = {}


def _pieces(rb):
    """Row pieces (chunk, lo, hi) outside the band, in rotated coords."""
    if rb == 0:
        out = [(0, BW - BOFF, CH)]
        out += [(c, 0, CH) for c in range(1, NCH - 1)]
        out.append((NCH - 1, 0, CH - BOFF))
        return out
    a = 128 * rb - BOFF
    b = 128 * rb + (BW - BOFF)
    out = [(0, 0, a), (0, b, CH)]
    out += [(c, 0, CH) for c in range(1, NCH)]
    return out


def _build_nc():
    nc = bacc.Bacc("TRN2", target_bir_lowering=False, debug=False)

    d_rhs2 = nc.dram_tensor("rhs2", [D, N], BF16, kind="ExternalInput").ap()
    d_lhsT = nc.dram_tensor("lhsT", [D, M], BF16, kind="ExternalInput").ap()
    d_auxr = nc.dram_tensor("auxr", [4, N], BF16, kind="ExternalInput").ap()
    d_auxl = nc.dram_tensor("auxl", [4, M], BF16, kind="ExternalInput").ap()
    d_brhs = nc.dram_tensor("brhs", [D, RB * BW], BF16, kind="ExternalInput").ap()
    d_baux = nc.dram_tensor("baux", [4, RB * BW], BF16, kind="ExternalInput").ap()
    d_ind = nc.dram_tensor("ind", [128, RB * BW], F32, kind="ExternalInput").ap()
    d_bigind = nc.dram_tensor("bigind", [128, RB * BW], F32, kind="ExternalInput").ap()
    d_w = nc.dram_tensor("w", [128, RB], F32, kind="ExternalInput").ap()
    d_out = nc.dram_tensor("out", [1, 1], F32, kind="ExternalOutput").ap()

    AL = mybir.AluOpType
    AX = mybir.AxisListType
    AF = mybir.ActivationFunctionType

    with tile.TileContext(nc) as tc:
        with (
            tc.tile_pool(name="const", bufs=1) as const,
            tc.tile_pool(name="rpool", bufs=2) as rpool,
            tc.tile_pool(name="ps", bufs=2, space="PSUM") as ps,
            tc.tile_pool(name="bnd", bufs=2) as bnd,
            tc.tile_pool(name="small", bufs=1) as small,
        ):
            t_lhsT = const.tile([D, M], BF16)
            t_auxr = const.tile([4, N], BF16)
            t_auxl = const.tile([4, M], BF16)
            t_brhs = const.tile([D, RB * BW], BF16)
            t_baux = const.tile([4, RB * BW], BF16)
            t_ind = const.tile([128, RB * BW], F32)
            t_bigind = const.tile([128, RB * BW], F32)
            t_w = const.tile([128, RB], F32)
            for t, d in [
                (t_lhsT, d_lhsT), (t_auxr, d_auxr), (t_auxl, d_auxl),
                (t_brhs, d_brhs), (t_baux, d_baux), (t_ind, d_ind),
                (t_bigind, d_bigind), (t_w, d_w),
            ]:
                nc.sync.dma_start(out=t[:], in_=d)

            permax = const.tile([128, RB * 8], F32)
            posacc = const.tile([128, RB], F32)
            negmax = const.tile([128, RB], F32)
            nc.vector.memset(permax[:], NEG_INF)

            piece_lists = [_pieces(rb) for rb in range(RB)]
            piece_cols = []  # per rb: next free col
            for rb in range(RB):
                piece_cols.append(0)

            # ---- negative mining: full row minus band, plain reduces ----
            for c in range(NCH):
                rt = rpool.tile([D, CH], BF16, tag="rhs")
                nc.sync.dma_start(out=rt[:], in_=d_rhs2[:, c * CH:(c + 1) * CH])
                for rb in range(RB):
                    pst = ps.tile([128, CH], F32, tag="ps")
                    for b in range(CH // 512):
                        sl = slice(b * 512, b * 512 + 512)
                        nc.tensor.matmul(
                            pst[:, sl],
                            lhsT=t_lhsT[:, rb * 128:rb * 128 + 128],
                            rhs=rt[:, sl],
                            start=True, stop=False,
                        )
                        nc.tensor.matmul(
                            pst[:, sl],
                            lhsT=t_auxl[:, rb * 128:rb * 128 + 128],
                            rhs=t_auxr[:, c * CH + b * 512:c * CH + b * 512 + 512],
                            start=False, stop=True,
                        )
                    for (pc, lo, hi) in piece_lists[rb]:
                        if pc != c:
                            continue
                        col = rb * 8 + piece_cols[rb]
                        piece_cols[rb] += 1
                        nc.vector.tensor_reduce(
                            permax[:, col:col + 1], pst[:, lo:hi],
                            axis=AX.X, op=AL.max,
                        )

            # ---- band: +d2 via small matmuls; masked pos/neg ----
            for rb in range(RB):
                bps = ps.tile([128, CH], F32, tag="ps")
                bsl = slice(rb * BW, (rb + 1) * BW)
                nc.tensor.matmul(
                    bps[:, 0:BW],
                    lhsT=t_lhsT[:, rb * 128:rb * 128 + 128],
                    rhs=t_brhs[:, bsl],
                    start=True, stop=False,
                )
                nc.tensor.matmul(
                    bps[:, 0:BW],
                    lhsT=t_auxl[:, rb * 128:rb * 128 + 128],
                    rhs=t_baux[:, bsl],
                    start=False, stop=True,
                )
                # neg: max over band of (-d2 - BIG*ind), fused
                bn2 = bnd.tile([128, BW], F32, tag="bn2")
                nc.vector.scalar_tensor_tensor(
                    bn2[:], bps[:, 0:BW], -1.0, t_bigind[:, bsl],
                    op0=AL.mult, op1=AL.subtract,
                )
                col = rb * 8 + piece_cols[rb]
                piece_cols[rb] += 1
                nc.vector.tensor_reduce(
                    permax[:, col:col + 1], bn2[:], axis=AX.X, op=AL.max
                )
                # pos: max over band of (d2 + BIG)*ind, -BIG fixup post-reduce
                bp2 = bnd.tile([128, BW], F32, tag="bp2")
                nc.vector.scalar_tensor_tensor(
                    bp2[:], bps[:, 0:BW], BIG, t_ind[:, bsl],
                    op0=AL.add, op1=AL.mult,
                )
                posraw = bnd.tile([128, 1], F32, tag="praw")
                nc.vector.tensor_reduce(
                    posraw[:], bp2[:], axis=AX.X, op=AL.max
                )
                nc.vector.tensor_scalar_add(posacc[:, rb:rb + 1], posraw[:], -BIG)

            # ---- combine piece maxima ----
            for rb in range(RB):
                nc.vector.tensor_reduce(
                    negmax[:, rb:rb + 1], permax[:, rb * 8:rb * 8 + 8],
                    axis=AX.X, op=AL.max,
                )

            # ---- tail ----
            d2n = small.tile([128, RB], F32)
            nc.vector.tensor_scalar_mul(d2n[:], negmax[:], -1.0)
            d2p_r = small.tile([128, RB], F32)
            d2n_r = small.tile([128, RB], F32)
            nc.scalar.activation(d2p_r[:], posacc[:], AF.Relu)
            nc.scalar.activation(d2n_r[:], d2n[:], AF.Relu)
            pd = small.tile([128, RB], F32)
            nd = small.tile([128, RB], F32)
            nc.scalar.activation(pd[:], d2p_r[:], AF.Sqrt)
            nc.scalar.activation(nd[:], d2n_r[:], AF.Sqrt)
            diff = small.tile([128, RB], F32)
            nc.vector.tensor_sub(diff[:], pd[:], nd[:])
            diffm = small.tile([128, RB], F32)
            nc.vector.tensor_scalar_add(diffm[:], diff[:], MARGIN)
            per = small.tile([128, RB], F32)
            nc.scalar.activation(per[:], diffm[:], AF.Relu)
            perw = small.tile([128, RB], F32)
            nc.vector.tensor_mul(perw[:], per[:], t_w[:])

            ones = small.tile([128, 1], F32)
            nc.vector.memset(ones[:], 1.0)
            sps = ps.tile([128, CH], F32, tag="ps")
            nc.tensor.matmul(sps[0:1, 0:RB], lhsT=ones[:], rhs=perw[:], start=True, stop=True)
            srow = small.tile([1, RB], F32)
            nc.vector.tensor_copy(srow[:], sps[0:1, 0:RB])
            tot = small.tile([1, 1], F32)
            nc.vector.tensor_reduce(tot[:], srow[:], axis=AX.X, op=AL.add)
            nc.sync.dma_start(out=d_out, in_=tot[:])

    nc.compile()
    return nc


def _prep(features, labels):
    lab = np.asarray(labels).astype(np.int64).ravel()
    X = np.asarray(features, dtype=np.float32)
    assert X.shape == (N, D) and lab.shape == (N,)

    order = np.argsort(lab, kind="stable")
    Xs = np.ascontiguousarray(X[order])
    ls = lab[order]
    S = np.searchsorted(ls, ls, side="left").astype(np.int64)
    E = np.searchsorted(ls, ls, side="right").astype(np.int64)
    csize = E - S
    assert csize.max() <= BOFF + 1, f"class too large: {csize.max()}"
    valid = (csize < N).astype(np.float32)

    sq = (Xs.astype(np.float64) ** 2).sum(1).astype(np.float32)
    sq_hi = sq.astype(BF).astype(np.float32)
    sq_lo = sq - sq_hi
    XT = np.ascontiguousarray(Xs.T)                      # [D, N] f32

    rhs2_full = (2.0 * XT).astype(BF)
    one = np.ones(N, np.float32)
    auxr_full = np.stack([-sq_hi, -sq_lo, -one, -one]).astype(BF)

    in_maps = []
    total_valid = float(valid.sum())
    for k in range(NCORES):
        a0 = k * M
        lhsT = XT[:, a0:a0 + M].astype(BF)
        onem = np.ones(M, np.float32)
        auxl = np.stack([onem, onem, sq_hi[a0:a0 + M], sq_lo[a0:a0 + M]]).astype(BF)
        rhs2 = np.roll(rhs2_full, -a0, axis=1)
        auxr = np.roll(auxr_full, -a0, axis=1)

        brhs = np.zeros((D, RB * BW), np.float32)
        baux = np.zeros((4, RB * BW), np.float32)
        ind = np.zeros((128, RB * BW), np.float32)
        for rb in range(RB):
            g0 = a0 + rb * 128
            aidx = g0 + np.arange(128)
            bb0 = g0 - BOFF
            cols = np.arange(bb0, bb0 + BW)
            v = (cols >= 0) & (cols < N)
            cv = cols[v]
            bsl = slice(rb * BW, (rb + 1) * BW)
            brhs[:, bsl][:, v] = -2.0 * XT[:, cv]
            ba = np.zeros((4, BW), np.float32)
            ba[0, v] = sq_hi[cv]
            ba[0, ~v] = PAD_SQ
            ba[1, v] = sq_lo[cv]
            ba[2, :] = 1.0
            ba[3, :] = 1.0
            baux[:, bsl] = ba
            lo = (S[aidx] - bb0)[:, None]
            hi = (E[aidx] - bb0)[:, None]
            assert (lo >= 0).all() and (hi <= BW).all()
            j = np.arange(BW)[None, :]
            ind[:, bsl] = ((j >= lo) & (j < hi)).astype(np.float32)

        wk = valid[a0:a0 + M].reshape(RB, 128).T.copy()
        in_maps.append({
            "rhs2": rhs2, "lhsT": lhsT, "auxr": auxr, "auxl": auxl,
            "brhs": brhs.astype(BF), "baux": baux.astype(BF),
            "ind": ind, "bigind": (BIG * ind).astype(np.float32), "w": wk,
        })
    return in_maps, total_valid


_NC_CACHE = None


def kernel(features, labels):
    global _NC_CACHE, LAST_RESULTS
    in_maps, total_valid = _prep(features, labels)
    if _NC_CACHE is None:
        _NC_CACHE = _build_nc()
    nc = _NC_CACHE
    res = run_bass_kernel_spmd(nc, in_maps, list(range(NCORES)), trace=TRACE)
    LAST_RESULTS = {"bass": res}
    s = sum(float(res.results[k]["out"][0, 0]) for k in range(NCORES))
    loss = s / total_valid if total_valid > 0 else 0.0
    return np.float32(loss)


if __name__ == "__main__":
    from concourse.bass_interp import CoreSim

    sys.path.insert(0, "/root/problem")
    import reference

    inputs = {k: np.asarray(v) for k, v in reference.setup_inputs().items()}
    in_maps, total_valid = _prep(inputs["features"], inputs["labels"])
    nc = _build_nc()
    core = int(sys.argv[1]) if len(sys.argv) > 1 else 0
    sim = CoreSim(nc)
    for k2, v in in_maps[core].items():
        sim.tensor(k2)[:] = v
    sim.simulate()
    got = float(np.array(sim.tensor("out"))[0, 0])

    lab = np.asarray(inputs["labels"]).astype(np.int64).ravel()
    X = np.asarray(inputs["features"], np.float32)
    order = np.argsort(lab, kind="stable")
    Xs, ls = X[order], lab[order]
    d2 = ((Xs[core * M:(core + 1) * M, None] - Xs[None, :, :]) ** 2).sum(-1)
    pos_mask = ls[None, :] == ls[core * M:(core + 1) * M, None]
    pm = np.where(pos_mask, d2, -np.inf).max(1)
    nm = np.where(~pos_mask, d2, np.inf).min(1)
    per = np.maximum(np.sqrt(np.maximum(pm, 0)) - np.sqrt(np.maximum(nm, 0)) + MARGIN, 0)
    expected = per.sum()
    print(f"core{core} partial: got {got:.6f} expected {expected:.6f} "
          f"rel {abs(got - expected) / max(abs(expected), 1e-9):.3e}")



# revision 7
# speedup vs baseline: 1.6197x; 1.6197x over previous
"""Batch-hard triplet loss (CrossCameraTripletLoss) on 8 TRN2 NeuronCores.

Strategy (data-parallel over anchor rows, label-sorted + per-core rotated):
  - Host: stable-sort rows by label so each class is a contiguous range
    [S, E); shard 1024 sorted anchors per core. Each core receives the full
    candidate set ROTATED left by its anchor offset, so row-block rb's class
    windows always live in the fixed column band [128*rb-64, 128*rb+192) -
    the same program works on every core (SPMD).
  - PE: fp8(e4m3) DoubleRow matmuls with K_eff=130: 64 partitions carry the
    128 feature dims as (d, d+64) pairs, partition 64 carries (-sq_hi,
    -sq_lo) norm rows against all-ones weights, so PSUM holds the corrected
    score c_ij = 2<x_i,x_j> - sq_j = sq_i - d2_ij directly. 512-out-col
    matmuls (1024 fp8 moving) at 0.5 cycles/col.
  - Mining: Act converts most PSUM tiles to fp16 SBUF (DVE then max-reduces
    at 4x fast mode); DVE max-reduces the rest straight from PSUM (1x).
    Band (class window, +-30000 fp16 mask M via tensor_tensor_reduce):
      neg: max (c - M)   (window suppressed)
      pos: max (M - c)   (out-of-window suppressed; = 30000 + max -c)
  - Output [128, 80] f32 per core: raw piece maxima; host combines, then
    neg_d2 = sq_i - negmax, pos_d2 = sq_i + posraw - 30000,
    loss_i = relu(sqrt(pos_d2) - sqrt(neg_d2) + margin); mean over anchors.
  - pairwise_distance's eps (1e-6 on the difference) shifts distances by
    ~1e-7 relative - far below tolerance - so mined values are used directly.
"""

import sys

sys.path.insert(0, "/opt/trn_rl_repo")

import numpy as np
import ml_dtypes

import concourse.bacc as bacc
import concourse.mybir as mybir
import concourse.tile as tile
from concourse.bass_utils import run_bass_kernel_spmd

F32 = mybir.dt.float32
F16 = mybir.dt.float16
FP8 = mybir.dt.float8e4
NP8 = ml_dtypes.float8_e4m3
MARGIN = 0.2
BIGM = 30000.0
NEG_INIT = -60000.0

N, D, NCORES = 8192, 128, 8
M = N // NCORES          # anchors per core
RB = M // 128            # row blocks per core
CH = 2048                # chunk width (4 PSUM banks)
NCH = N // CH
BW = 256                 # band width
BOFF = 64                # band margin left of the block's first anchor
KP = D // 2 + 1          # 65 partitions: 64 feature pairs + 1 norm row

NPM = 8                  # permax cols per rb
NPP = 2                  # pos cols per rb

TRACE = False
LAST_RESULTS = {}


def _pieces(rb, c):
    """Column ranges of chunk c (local coords) outside rb's band."""
    if rb == 0:
        if c == 0:
            return [(192, CH)]
        if c == NCH - 1:
            return [(0, CH - BOFF)]
        return [(0, CH)]
    if c == 0:
        a, b = 128 * rb - BOFF, 128 * rb + (BW - BOFF)
        return [(0, a), (b, CH)]
    return [(0, CH)]


def _band_parts(rb, c):
    """(lo, hi, mask_off) pieces of rb's band inside chunk c."""
    if rb == 0:
        if c == 0:
            return [(0, 192, 64)]
        if c == NCH - 1:
            return [(CH - BOFF, CH, 0)]
        return []
    if c == 0:
        return [(128 * rb - BOFF, 128 * rb + (BW - BOFF), 0)]
    return []


# tiles (rb, c) whose PSUM is converted to fp16 by Act (rest: DVE direct).
# band tiles (c==0 and (0, NCH-1)) must be converted; add others for balance.
def _convert(rb, c):
    if c == 0 or (rb == 0 and c == NCH - 1):
        return True
    return c in (1, 2)  # 8 + 1 + 16 = 25 converted, 7 direct


def _build_nc():
    nc = bacc.Bacc("TRN2", target_bir_lowering=False, debug=False)

    d_rhs = nc.dram_tensor("rhs", [KP, 2, N], FP8, kind="ExternalInput").ap()
    d_lhsT = nc.dram_tensor("lhsT", [KP, 2, M], FP8, kind="ExternalInput").ap()
    d_mask = nc.dram_tensor("mask", [128, RB * BW], F16, kind="ExternalInput").ap()
    d_out = nc.dram_tensor(
        "out", [128, RB * (NPM + NPP)], F32, kind="ExternalOutput"
    ).ap()

    AL = mybir.AluOpType
    AX = mybir.AxisListType
    AF = mybir.ActivationFunctionType
    DR = mybir.MatmulPerfMode.DoubleRow

    with tile.TileContext(nc) as tc:
        with (
            tc.tile_pool(name="const", bufs=1) as const,
            tc.tile_pool(name="ps", bufs=2, space="PSUM") as ps,
            tc.tile_pool(name="cf", bufs=3) as cfp,
            tc.tile_pool(name="sc", bufs=2) as scp,
            tc.tile_pool(name="small", bufs=1) as small,
        ):
            t_rhs = const.tile([KP, 2, N], FP8)
            t_lhsT = const.tile([KP, 2, M], FP8)
            t_mask = const.tile([128, RB * BW], F16)
            for c in range(NCH):
                nc.sync.dma_start(
                    out=t_rhs[:, :, c * CH:(c + 1) * CH],
                    in_=d_rhs[:, :, c * CH:(c + 1) * CH],
                )
            nc.sync.dma_start(out=t_lhsT[:], in_=d_lhsT)
            nc.sync.dma_start(out=t_mask[:], in_=d_mask)

            permax = small.tile([128, RB * NPM], F32)
            posp = small.tile([128, RB * NPP], F32)
            nc.vector.memset(permax[:], NEG_INIT)
            nc.vector.memset(posp[:], NEG_INIT)

            ncol = [0] * RB

            for rb in range(RB):
                for c in range(NCH):
                    pst = ps.tile([128, CH], F32, tag="ps")
                    for b in range(CH // 512):
                        sl = slice(b * 512, b * 512 + 512)
                        nc.tensor.matmul(
                            pst[:, sl],
                            lhsT=t_lhsT[:, :, rb * 128:rb * 128 + 128],
                            rhs=t_rhs[:, :, c * CH + b * 512:c * CH + b * 512 + 512],
                            start=True, stop=True,
                            perf_mode=DR,
                        )
                    if _convert(rb, c):
                        cf = cfp.tile([128, CH], F16, tag="cf")
                        nc.scalar.activation(cf[:], pst[:], AF.Copy)
                        src = cf
                    else:
                        src = pst
                    for (lo, hi) in _pieces(rb, c):
                        col = rb * NPM + ncol[rb]
                        ncol[rb] += 1
                        nc.vector.tensor_reduce(
                            permax[:, col:col + 1], src[:, lo:hi],
                            axis=AX.X, op=AL.max,
                        )
                    for (lo, hi, moff) in _band_parts(rb, c):
                        w = hi - lo
                        msl = slice(rb * BW + moff, rb * BW + moff + w)
                        col = rb * NPM + ncol[rb]
                        ncol[rb] += 1
                        scn = scp.tile([128, BW], F16, tag="scn")
                        nc.vector.scalar_tensor_tensor(
                            scn[:, 0:w], cf[:, lo:hi], 1.0, t_mask[:, msl],
                            op0=AL.mult, op1=AL.subtract,
                        )
                        nc.vector.tensor_reduce(
                            permax[:, col:col + 1], scn[:, 0:w],
                            axis=AX.X, op=AL.max,
                        )
                        pcol = rb * NPP + (1 if (rb == 0 and c == NCH - 1) else 0)
                        scq = scp.tile([128, BW], F16, tag="scq")
                        nc.vector.scalar_tensor_tensor(
                            scq[:, 0:w], cf[:, lo:hi], -1.0, t_mask[:, msl],
                            op0=AL.mult, op1=AL.add,
                        )
                        nc.vector.tensor_reduce(
                            posp[:, pcol:pcol + 1], scq[:, 0:w],
                            axis=AX.X, op=AL.max,
                        )

            out_t = small.tile([128, RB * (NPM + NPP)], F32)
            nc.vector.tensor_copy(out_t[:, 0:RB * NPM], permax[:])
            nc.vector.tensor_copy(
                out_t[:, RB * NPM:RB * (NPM + NPP)], posp[:]
            )
            nc.sync.dma_start(out=d_out, in_=out_t[:])

    nc.compile()
    return nc


def _q8(x):
    return np.asarray(x).astype(NP8)


def _prep(features, labels):
    lab = np.asarray(labels).astype(np.int64).ravel()
    X = np.asarray(features, dtype=np.float32)
    assert X.shape == (N, D) and lab.shape == (N,)

    order = np.argsort(lab, kind="stable")
    Xs = np.ascontiguousarray(X[order])
    ls = lab[order]
    S = np.searchsorted(ls, ls, side="left").astype(np.int64)
    E = np.searchsorted(ls, ls, side="right").astype(np.int64)
    csize = E - S
    assert csize.max() <= BOFF + 1, f"class too large: {csize.max()}"

    sq = (Xs.astype(np.float64) ** 2).sum(1).astype(np.float32)
    sq_hi = _q8(sq).astype(np.float32)
    sq_lo = sq - sq_hi

    # fp8 DoubleRow layouts: [KP, 2, cols]
    X8 = _q8(Xs)                      # [N, D] anchors (weights side)
    R8 = _q8(2.0 * Xs)                # [N, D] candidates (moving side)

    rhs_full = np.zeros((KP, 2, N), NP8)
    rhs_full[:64, 0, :] = R8[:, 0:64].T
    rhs_full[:64, 1, :] = R8[:, 64:128].T
    rhs_full[64, 0, :] = _q8(-sq_hi)
    rhs_full[64, 1, :] = _q8(-sq_lo)

    in_maps = []
    for k in range(NCORES):
        a0 = k * M
        lhsT = np.zeros((KP, 2, M), NP8)
        lhsT[:64, 0, :] = X8[a0:a0 + M, 0:64].T
        lhsT[:64, 1, :] = X8[a0:a0 + M, 64:128].T
        lhsT[64, :, :] = np.ones((2, M), NP8)
        rhs = np.roll(rhs_full, -a0, axis=2)

        mask = np.zeros((128, RB * BW), np.float16)
        for rb in range(RB):
            g0 = a0 + rb * 128
            aidx = g0 + np.arange(128)
            bb0 = g0 - BOFF
            lo = (S[aidx] - bb0)[:, None]
            hi = (E[aidx] - bb0)[:, None]
            assert (lo >= 0).all() and (hi <= BW).all()
            j = np.arange(BW)[None, :]
            mask[:, rb * BW:(rb + 1) * BW] = np.where(
                (j >= lo) & (j < hi), np.float16(BIGM), np.float16(0.0)
            )
        in_maps.append({
            "rhs": np.ascontiguousarray(rhs),
            "lhsT": np.ascontiguousarray(lhsT),
            "mask": mask,
        })
    return in_maps, sq


def _postprocess(outs, sq):
    """outs: list of [128, RB*(NPM+NPP)] f32 per core; sq: [N] sorted norms."""
    per_anchor = []
    for k in range(NCORES):
        o = np.asarray(outs[k], np.float32)
        permax = o[:, 0:RB * NPM].reshape(128, RB, NPM)
        posp = o[:, RB * NPM:].reshape(128, RB, NPP)
        negmax = permax.max(axis=2)      # [p, rb]
        posraw = posp.max(axis=2)
        a0 = k * M
        g = a0 + np.arange(RB)[None, :] * 128 + np.arange(128)[:, None]
        sqa = sq[g]                      # [p, rb]
        negd2 = np.maximum(sqa - negmax, 0.0)
        posd2 = np.maximum(sqa + (posraw - BIGM), 0.0)
        per = np.maximum(np.sqrt(posd2) - np.sqrt(negd2) + MARGIN, 0.0)
        per_anchor.append(per.ravel())
    allv = np.concatenate(per_anchor)
    return float(allv.mean())


_NC_CACHE = None


def kernel(features, labels):
    global _NC_CACHE, LAST_RESULTS
    in_maps, sq = _prep(features, labels)
    if _NC_CACHE is None:
        _NC_CACHE = _build_nc()
    nc = _NC_CACHE
    res = run_bass_kernel_spmd(nc, in_maps, list(range(NCORES)), trace=TRACE)
    LAST_RESULTS = {"bass": res}
    loss = _postprocess([res.results[k]["out"] for k in range(NCORES)], sq)
    return np.float32(loss)


if __name__ == "__main__":
    from concourse.bass_interp import CoreSim

    sys.path.insert(0, "/root/problem")
    import reference

    inputs = {k: np.asarray(v) for k, v in reference.setup_inputs().items()}
    in_maps, sq = _prep(inputs["features"], inputs["labels"])
    nc = _build_nc()
    core = int(sys.argv[1]) if len(sys.argv) > 1 else 0
    sim = CoreSim(nc)
    for k2, v in in_maps[core].items():
        sim.tensor(k2)[:] = v
    sim.simulate()
    o = np.array(sim.tensor("out"))

    # numpy replica of the mining for this core
    lab = np.asarray(inputs["labels"]).astype(np.int64).ravel()
    X = np.asarray(inputs["features"], np.float32)
    order = np.argsort(lab, kind="stable")
    Xs, ls = X[order], lab[order]
    d2 = ((Xs[core * M:(core + 1) * M, None] - Xs[None, :, :]) ** 2).sum(-1)
    pos_mask = ls[None, :] == ls[core * M:(core + 1) * M, None]
    pm = np.where(pos_mask, d2, -np.inf).max(1)
    nm = np.where(~pos_mask, d2, np.inf).min(1)
    per_ref = np.maximum(
        np.sqrt(np.maximum(pm, 0)) - np.sqrt(np.maximum(nm, 0)) + MARGIN, 0
    )

    permax = o[:, 0:RB * NPM].reshape(128, RB, NPM)
    posp = o[:, RB * NPM:].reshape(128, RB, NPP)
    negmax = permax.max(axis=2)
    posraw = posp.max(axis=2)
    a0 = core * M
    g = a0 + np.arange(RB)[None, :] * 128 + np.arange(128)[:, None]
    sqa = sq[g]
    negd2 = np.maximum(sqa - negmax, 0.0)
    posd2 = np.maximum(sqa + (posraw - BIGM), 0.0)
    per = np.maximum(np.sqrt(posd2) - np.sqrt(negd2) + MARGIN, 0.0)
    per_dev = np.zeros(M)
    for rb in range(RB):
        per_dev[rb * 128:(rb + 1) * 128] = per[:, rb]
    err = np.abs(per_dev - per_ref)
    print(f"core{core}: sum dev {per_dev.sum():.6f} ref {per_ref.sum():.6f} "
          f"max per-anchor err {err.max():.4f} mean {err.mean():.5f}")
